# revision 40
# baseline (speedup 1.0000x reference)
"""Trainium2 Bass kernel for nn_Attention_27994596836196.

GQA attention block (B=2, S=2048, HID=4096, 32 q heads / 8 kv groups,
rope, causal, out-projection), tensor-parallel over the 8 NeuronCores of
one TRN2 chip: core c owns q heads 4c..4c+3 and kv group c.  Each core
computes its heads' Q^T/K^T/V projections from a host-pretransposed
activation matrix, runs causal attention in a transposed
(keys-on-partitions) layout, and contracts its 512-row slice of w_o into
a full-size partial output; the host sums the eight partials
(collectives deliberately avoided: a collective in the NEFF measurably
slows every PE instruction by ~20%).

v2 optimizations over the 849us baseline:
  - softmax denominator accumulated on DVE (tensor_add over exp tiles)
    instead of ones-matmuls on PE (-70us PE work)
  - causal diagonal tiles stream only their valid column range
  - V transposed on-chip via PE identity-matmuls (no DRAM roundtrip, no
    DMA_TRANSPOSE serialization)
  - software pipelining: b0 attention interleaves with b1 projection
    granules; b1 attention interleaves with b0 w_o emission, so PE has
    matmul filler while ACT runs exps
  - startup DMAs staggered (x chunk 0 in quarters, K projection first)
  - w_o PSUM evictions rotate across DVE/ACT/Pool engines

Self-contained: builds and runs via concourse (bass/tile) from
/opt/trn_rl_repo through bass_utils.run_bass_kernel_spmd on cores 0-7.
"""

import os
import sys
from collections import deque

sys.path.insert(0, "/opt/trn_rl_repo")

import numpy as np
import ml_dtypes

# NTFF profiling hook shim: this agent image's antenv package lacks
# axon_hooks, which run_bass_kernel_spmd(trace=True) imports.  Harmless
# when tracing is off; registers the real hook when available.
try:
    import antenv.axon_hooks  # noqa: F401
except ImportError:
    import types

    _m = types.ModuleType("antenv.axon_hooks")
    _m._HOOK = None
    _m.set_axon_ntff_profile_hook = lambda h: setattr(_m, "_HOOK", h)
    _m.get_axon_ntff_profile_hook = lambda: _m._HOOK
    sys.modules["antenv.axon_hooks"] = _m
    try:
        import antenv

        antenv.axon_hooks = _m
        from trn_agent_boot.trn_boot import _ntff_profile_via_ctypes

        _m.set_axon_ntff_profile_hook(
            _ntff_profile_via_ctypes("/opt/axon/libaxon_pjrt.so")
        )
    except Exception:
        pass

import bass_rust
import concourse.bass as bass
import concourse.tile as tile
from concourse import mybir
from concourse.bass_utils import run_bass_kernel_spmd
from contextlib import ExitStack

# ---------------------------------------------------------------------------
# Workaround for this walrus build's cap of ONE sync-wait command per
# instruction: Tile's sem-assignment freely attaches several waits to one
# instruction and codegen rejects it ("Too many sync wait commands").
# Split the waits across same-engine NoOps preceding the instruction.
# ---------------------------------------------------------------------------
MAX_WAITS = 1


def split_multi_waits(nc):
    n_split = 0
    for f in nc.m.functions:
        for bb in f.blocks:
            out = []
            for inst in bb.instructions:
                si = inst.sync_info
                if si is not None and si.on_wait and len(si.on_wait) > MAX_WAITS:
                    waits = list(si.on_wait)
                    extra, keep = waits[:-MAX_WAITS], waits[-MAX_WAITS:]
                    for i in range(0, len(extra), MAX_WAITS):
                        nop = bass_rust.InstNoOp(
                            name=f"I-{nc.next_id()}", ins=[], outs=[]
                        )
                        nop.engine = inst.engine
                        nop.sync_info = mybir.SyncInfo(
                            on_wait=extra[i : i + MAX_WAITS], on_update=[]
                        )
                        out.append(nop)
                    si.on_wait = keep
                    n_split += 1
                out.append(inst)
            bb.instructions[:] = out
    return n_split


BF16 = mybir.dt.bfloat16
F32 = mybir.dt.float32

N_CORES = 8
B, S, HID = 2, 2048, 4096
BS = B * S  # 4096
D = 128
NH = 4          # q heads per core
KT = HID // 128  # 32 k-tiles
SC = 512        # free-dim chunk
NSC = BS // SC  # 8
SCALE = 1.0 / (D ** 0.5)
EXP = mybir.ActivationFunctionType.Exp
LOG = mybir.ActivationFunctionType.Ln

# projection target order: K first (smallest weight DMA gates startup),
# then the 4 Q heads, V last (V has no rope; its eviction transposes).
TGT = [4, 0, 1, 2, 3, 5]


def build():
    nc = bass.Bass(num_devices=N_CORES)

    # all inputs host-prearranged to partition-major layouts so every DMA
    # is 128 large contiguous descriptors (descriptor-rate gated startup
    # otherwise: gather loads cost 2-9us EACH to issue)
    xC = nc.declare_dram_parameter("xC", [128, NSC, KT, SC], BF16, isOutput=False)
    wq = nc.declare_dram_parameter("wq", [128, KT, NH * D], BF16, isOutput=False)
    wk = nc.declare_dram_parameter("wk", [128, KT, D], BF16, isOutput=False)
    wv = nc.declare_dram_parameter("wv", [128, KT, D], BF16, isOutput=False)
    wo = nc.declare_dram_parameter("wo", [128, NH, HID], BF16, isOutput=False)
    cosF = nc.declare_dram_parameter("cosF", [D, BS], BF16, isOutput=False)
    sinF = nc.declare_dram_parameter("sinF", [D, BS], BF16, isOutput=False)
    swapP = nc.declare_dram_parameter("swapP", [D, D], BF16, isOutput=False)
    ident = nc.declare_dram_parameter("ident", [D, D], BF16, isOutput=False)
    maskT = nc.declare_dram_parameter("maskT", [D, D], BF16, isOutput=False)
    outT = nc.declare_dram_parameter("outT", [HID, BS], BF16, isOutput=True)

    with tile.TileContext(nc, num_cores=N_CORES) as tc, ExitStack() as ctx:
        # ---- long-lived pools -------------------------------------------
        singles = ctx.enter_context(tc.tile_pool(name="singles", bufs=1))
        qkv = ctx.enter_context(tc.tile_pool(name="qkv", bufs=1))
        pexp = ctx.enter_context(tc.tile_pool(name="pexp", bufs=8))
        lpool = ctx.enter_context(tc.tile_pool(name="lp", bufs=2))
        rlg = ctx.enter_context(tc.tile_pool(name="rlg", bufs=1))
        rrec = ctx.enter_context(tc.tile_pool(name="rrec", bufs=2))
        # PSUM: 8 banks of [128,512]f32 exactly:
        ps_p = ctx.enter_context(tc.tile_pool(name="ps_p", bufs=2, space="PSUM"))
        ps_s = ctx.enter_context(tc.tile_pool(name="ps_s", bufs=2, space="PSUM"))
        ps_o = ctx.enter_context(tc.tile_pool(name="ps_o", bufs=2, space="PSUM"))
        ps_x = ctx.enter_context(tc.tile_pool(name="ps_x", bufs=2, space="PSUM"))

        q_sb = [
            [
                qkv.tile([D, S], BF16, tag=f"q{b}_{h}", name=f"q_sb{b}_{h}")
                for h in range(NH)
            ]
            for b in range(B)
        ]
        k_sb = [qkv.tile([D, S], BF16, tag=f"k{b}", name=f"k_sb{b}") for b in range(B)]
        v_sb = [qkv.tile([D, S], BF16, tag=f"v{b}", name=f"v_sb{b}") for b in range(B)]

        # =================================================================
        # projection closures (stage 1 runs them inline; stage 2 pulls
        # them as PE filler between attention bursts)
        # =================================================================
        aall0 = ctx.enter_context(tc.tile_pool(name="aall0", bufs=1))
        ctx1 = ctx.enter_context(ExitStack())
        w1 = ctx1.enter_context(tc.tile_pool(name="w1", bufs=1))
        xtp = ctx1.enter_context(tc.tile_pool(name="xt", bufs=2))
        rope = ctx1.enter_context(tc.tile_pool(name="rope", bufs=3))
        vtp = ctx1.enter_context(tc.tile_pool(name="vt", bufs=2))

        wk_g = [
            w1.tile([128, 8, D], BF16, tag=f"wk{g}", name=f"wk_g{g}")
            for g in range(4)
        ]
        wv_sb = w1.tile([128, KT, D], BF16, tag="wv")
        wq_g = [
            w1.tile([128, 8, NH * D], BF16, tag=f"wq{g}", name=f"wq_g{g}")
            for g in range(4)
        ]
        cos_sb = singles.tile([D, BS], BF16)
        sin_sb = singles.tile([D, BS], BF16)
        mask_sb = singles.tile([D, D], BF16)
        swap_sb = singles.tile([D, D], BF16)
        ident_sb = singles.tile([D, D], BF16)
        ones_sb = singles.tile([D, D], BF16)
        nc.vector.memset(ones_sb[:], 1.0)

        def load_weights_early():
            # issued AFTER the first x piece + wk (HBM bw is the startup
            # gate: 8 cores pull simultaneously)
            nc.sync.dma_start(wq_g[0][:], wq[:, 0:8, :])
            nc.sync.dma_start(swap_sb[:], swapP[:])
            nc.sync.dma_start(ident_sb[:], ident[:])
            nc.sync.dma_start(mask_sb[:], maskT[:])
            nc.sync.dma_start(wq_g[1][:], wq[:, 8:16, :])
            nc.sync.dma_start(wq_g[2][:], wq[:, 16:24, :])
            nc.sync.dma_start(wq_g[3][:], wq[:, 24:32, :])
            nc.sync.dma_start(wv_sb[:], wv[:])
            nc.sync.dma_start(cos_sb[:], cosF[:])
            nc.sync.dma_start(sin_sb[:], sinF[:])

        pstate = {"pending": None, "xts": {}, "ps": {}}

        def rope_b(qeo, dst, sc):
            cols = bass.ts(sc, SC)
            loc = bass.ts(sc % 4, SC)
            ps_sw = ps_x.tile([D, SC], F32, tag="aux")
            nc.tensor.matmul(ps_sw[:], swap_sb[:], qeo[:], start=True, stop=True)
            t1 = rope.tile([D, SC], BF16, tag="t1")
            nc.vector.tensor_mul(t1[:], qeo[:], cos_sb[:, cols])
            t2 = rope.tile([D, SC], BF16, tag="t2")
            nc.vector.tensor_mul(t2[:], ps_sw[:], sin_sb[:, cols])
            nc.vector.tensor_add(dst[:, loc], t1[:], t2[:])

        def proj_dma(sc, pieces):
            ts = []
            for g in range(2):
                t = xtp.tile([128, KT // 2, SC], BF16, tag="xt")
                for q in range(pieces):
                    lo, hi = q * 16 // pieces, (q + 1) * 16 // pieces
                    nc.sync.dma_start(
                        t[:, lo:hi, :],
                        xC[:, sc, g * 16 + lo : g * 16 + hi, :],
                    )
                ts.append(t)
            pstate["xts"][sc] = ts

        def proj_half(sc, ti, half):
            tgt = TGT[ti]
            xth = pstate["xts"][sc]
            if half == 0:
                pstate["ps"][sc] = ps_t = ps_p.tile(
                    [D, SC], F32, tag="pp", name=f"pj{sc}_{ti}"
                )
            else:
                ps_t = pstate["ps"][sc]
            for k in range(half * 16, half * 16 + 16):
                if tgt == 4:
                    lhs = wk_g[k // 8][:, k % 8, :]
                elif tgt == 5:
                    lhs = wv_sb[:, k, :]
                else:
                    lhs = wq_g[k // 8][:, k % 8, bass.ts(tgt, D)]
                nc.tensor.matmul(
                    ps_t[:], lhs, xth[k // 16][:, k % 16, :],
                    start=(k == 0), stop=(k == KT - 1),
                )
            if half == 0:
                return
            b = sc // 4
            if tgt != 5:
                qeo = rope.tile([D, SC], BF16, tag="qeo")
                nc.vector.tensor_copy(qeo[:], ps_t[:])
                if pstate["pending"] is not None:
                    rope_b(*pstate["pending"])
                dst = k_sb[b] if tgt == 4 else q_sb[b][tgt]
                pstate["pending"] = (qeo, dst, sc)
            else:
                # V: PSUM -> SBUF (v^T), 4 PE transposes, Pool evict to
                # natural [sk, d] layout
                vt = vtp.tile([D, SC], BF16, tag="vt")
                nc.vector.tensor_copy(vt[:], ps_t[:])
                tp = ps_x.tile([D, SC], BF16, tag="aux", name=f"vt{sc}")
                for u in range(4):
                    nc.tensor.transpose(
                        tp[:, bass.ts(u, D)], vt[:, bass.ts(u, D)], ident_sb[:]
                    )
                nc.vector.tensor_copy(v_sb[b][:, bass.ts(sc % 4, SC)], tp[:])
                if pstate["pending"] is not None:
                    rope_b(*pstate["pending"])
                    pstate["pending"] = None

        def proj_closures(scs, skip_first_dma=False):
            cl = deque()
            for i, sc in enumerate(scs):
                if i == 0 and not skip_first_dma:
                    cl.append(lambda sc=sc: proj_dma(sc, 1))
                for ti in range(6):
                    for half in range(2):
                        if i + 1 < len(scs) and ti == 0 and half == 1:
                            def withdma(sc=sc, ti=ti, half=half, nx=scs[i + 1]):
                                proj_half(sc, ti, half)
                                proj_dma(nx, 1)
                            cl.append(withdma)
                        else:
                            cl.append(
                                lambda sc=sc, ti=ti, half=half: proj_half(
                                    sc, ti, half
                                )
                            )
            return cl

        # =================================================================
        # attention unit (b, h, c): 512 queries vs all causal keys.
        # keys-on-partitions layout; diagonal k-tiles stream only their
        # valid q-range; denominator accumulated on DVE; filler() is
        # pulled between QK quads to keep PE busy while ACT runs exps.
        # =================================================================
        evcnt = [0]

        def attn_unit(b, h, c, filler):
            nsk = 4 * (c + 1)
            j_seq = [(4 * c + dd, D * dd) for dd in range(4)] + [
                (j, 0) for j in range(4 * c)
            ]
            qh = q_sb[b][h]
            l_acc = lpool.tile([D, SC], BF16, tag="l")
            o_ps = ps_o.tile([D, SC], F32, tag="o")

            def av(idx, j, off, p):
                nc.tensor.matmul(
                    o_ps[:, off:],
                    v_sb[b][:, bass.ts(j, D)],
                    p[:, off:],
                    start=(idx == 0), stop=(idx == nsk - 1),
                    skip_group_check=(c == 0),
                )

            pend = deque()
            for idx, (j, off) in enumerate(j_seq):
                s_ps = ps_s.tile([D, SC], F32, tag="s")
                nc.tensor.matmul(
                    s_ps[:, off:],
                    k_sb[b][:, bass.ts(j, D)],
                    qh[:, c * SC + off : (c + 1) * SC],
                    start=True, stop=True,
                )
                p = pexp.tile([D, SC], BF16, tag="p")
                nc.scalar.activation(p[:, off:], s_ps[:, off:], EXP, scale=SCALE)
                if idx < 4:
                    nc.vector.tensor_mul(
                        p[:, off : off + D], p[:, off : off + D], mask_sb[:]
                    )
                if idx == 0:
                    nc.vector.tensor_copy(l_acc[:], p[:])
                else:
                    nc.vector.tensor_add(
                        l_acc[:, off:], l_acc[:, off:], p[:, off:]
                    )
                pend.append((idx, j, off, p))
                if len(pend) > 5:
                    av(*pend.popleft())
                if idx % 4 == 3:
                    filler()
            while pend:
                av(*pend.popleft())
            # partition-reduce the per-partition partial sums into the true
            # denominator, broadcast across partitions (single 512-stream
            # matmul per 512-query chunk instead of one per k-tile)
            l_bc = ps_s.tile([D, SC], F32, tag="s", name=f"lbc{b}_{h}_{c}")
            nc.tensor.matmul(l_bc[:], ones_sb[:], l_acc[:], start=True, stop=True)
            lg = rlg.tile([D, SC], F32, tag="lg")
            nc.scalar.activation(lg[:], l_bc[:], LOG)
            rec = rrec.tile([D, SC], F32, tag="rec")
            nc.scalar.activation(rec[:], lg[:], EXP, scale=-1.0)
            nc.vector.tensor_mul(
                a_all[b][h][:, bass.ts(c, SC)], o_ps[:], rec[:]
            )
            filler()

        # ---- w_o emission ------------------------------------------------
        def wo_pair(b, nl, m0, dve_only=False):
            for m in (m0, m0 + 1):
                pool, tg = (ps_p, "pp") if m % 2 == 0 else (ps_x, "aux")
                o_ps = pool.tile([D, SC], F32, tag=tg, name=f"wo{b}_{nl}_{m}")
                for hh in range(NH):
                    nc.tensor.matmul(
                        o_ps[:],
                        wo_h[m // 16][:, hh, bass.ts(m % 16, D)],
                        a_all[b][hh][:, bass.ts(nl, SC)],
                        start=(hh == 0), stop=(hh == NH - 1),
                    )
                ot = o3p.tile([D, SC], BF16, tag="ot")
                e = evcnt[0] = evcnt[0] + 1
                # ACT evictions only when no attention is interleaved —
                # they otherwise delay the exp chain and stall AV matmuls
                if dve_only or e % 2 == 0:
                    nc.vector.tensor_copy(ot[:], o_ps[:])
                else:
                    nc.scalar.copy(ot[:], o_ps[:])
                nc.sync.dma_start(
                    outT[bass.ts(m, D), b * S + nl * SC : b * S + (nl + 1) * SC],
                    ot[:],
                )

        a_all = [
            [
                aall0.tile([D, S], BF16, tag=f"a0_{h}", name=f"a_all0_{h}")
                for h in range(NH)
            ],
            None,  # batch-1 tiles allocated at stage 3 (SBUF lifetime)
        ]

        def make_pull(cl):
            def pull():
                if cl:
                    cl.popleft()()
            return pull

        # ---- stage 1: projections for batch 0 (sc 0-3) ------------------
        # first x quarter + wk lead the DMA queue: they gate the first
        # matmul, everything else streams behind them
        ts0 = [
            xtp.tile([128, KT // 2, SC], BF16, tag="xt", name=f"xt0_{g}")
            for g in range(2)
        ]
        pstate["xts"][0] = ts0
        nc.sync.dma_start(ts0[0][:, 0:2, :], xC[:, 0, 0:2, :])
        nc.sync.dma_start(wk_g[0][:], wk[:, 0:8, :])
        nc.sync.dma_start(ts0[0][:, 2:4, :], xC[:, 0, 2:4, :])
        nc.sync.dma_start(ts0[0][:, 4:8, :], xC[:, 0, 4:8, :])
        nc.sync.dma_start(wk_g[1][:], wk[:, 8:16, :])
        nc.sync.dma_start(wk_g[2][:], wk[:, 16:24, :])
        nc.sync.dma_start(wk_g[3][:], wk[:, 24:32, :])
        for q in range(2, 4):
            nc.sync.dma_start(ts0[0][:, 4 * q : 4 * q + 4, :], xC[:, 0, 4 * q : 4 * q + 4, :])
        for q in range(4):
            nc.sync.dma_start(
                ts0[1][:, 4 * q : 4 * q + 4, :], xC[:, 0, 16 + 4 * q : 16 + 4 * q + 4, :]
            )
        load_weights_early()
        for fn in proj_closures([0, 1, 2, 3], skip_first_dma=True):
            fn()

        # ---- stage 2: b1 projections interleaved with b0 attention ------
        cl2 = proj_closures([4, 5, 6, 7])
        pull2 = make_pull(cl2)
        for h in range(NH):
            for c in range(4):
                attn_unit(0, h, c, pull2)
        while cl2:
            pull2()
        ctx1.close()  # free w1/xt/rope/vt SBUF for stage 3

        # ---- stage 3: b1 attention interleaved with b0 w_o --------------
        w3 = ctx.enter_context(tc.tile_pool(name="w3", bufs=1))
        aall1 = ctx.enter_context(tc.tile_pool(name="aall1", bufs=1))
        o3p = ctx.enter_context(tc.tile_pool(name="o3p", bufs=6))
        a_all[1] = [
            aall1.tile([D, S], BF16, tag=f"a1_{h}", name=f"a_all1_{h}")
            for h in range(NH)
        ]
        # two halves so the first w_o fillers wait on 2MB, not 4MB
        wo_h = [
            w3.tile([128, NH, HID // 2], BF16, tag=f"wo{i}", name=f"wo_h{i}")
            for i in range(2)
        ]
        nc.gpsimd.dma_start(wo_h[0][:], wo[:, :, 0 : HID // 2])
        nc.gpsimd.dma_start(wo_h[1][:], wo[:, :, HID // 2 : HID])

        cl3 = deque()
        for nl in range(4):
            for m0 in range(0, KT, 2):
                cl3.append(lambda nl=nl, m0=m0: wo_pair(0, nl, m0))
        pull3 = make_pull(cl3)
        for h in range(NH):
            for c in range(4):
                attn_unit(1, h, c, pull3)
        while cl3:
            pull3()

        # ---- stage 4: b1 w_o --------------------------------------------
        for nl in range(4):
            for m0 in range(0, KT, 2):
                wo_pair(1, nl, m0)

    split_multi_waits(nc)
    return nc


BF16_NP = ml_dtypes.bfloat16


def prep_inputs(x, cos_half, sin_half, w_q, w_k, w_v, w_o):
    x = np.asarray(x)
    cos_half = np.asarray(cos_half, dtype=np.float32)
    sin_half = np.asarray(sin_half, dtype=np.float32)
    w_q, w_k, w_v, w_o = (np.asarray(a) for a in (w_q, w_k, w_v, w_o))

    X = x.reshape(B * S, HID)
    # partition-major x: xC[p, sc, k, s'] = X[sc*512+s', k*128+p]
    xC = np.ascontiguousarray(
        X.reshape(NSC, SC, KT, 128).transpose(3, 0, 2, 1)
    )

    cosb = cos_half.astype(BF16_NP)  # reference casts cos/sin to bf16 in _rope
    sinb = sin_half.astype(BF16_NP)
    cosF = np.tile(np.repeat(cosb.T, 2, axis=0), (1, B))  # (128, BS)
    sign = np.where(np.arange(D) % 2 == 0, -1.0, 1.0).astype(np.float32)
    sinF = np.tile(np.repeat(sinb.T, 2, axis=0) * sign[:, None].astype(BF16_NP), (1, B))
    cosF = np.ascontiguousarray(cosF, dtype=BF16_NP)
    sinF = np.ascontiguousarray(sinF, dtype=BF16_NP)

    swapP = np.zeros((D, D), dtype=BF16_NP)
    for i in range(D):
        swapP[i, i ^ 1] = 1.0

    ident = np.eye(D, dtype=BF16_NP)
    # strip mask for diagonal k-tiles: valid iff q_local >= k_local
    p = np.arange(D)[:, None]
    f = np.arange(D)[None, :]
    maskT = (f >= p).astype(BF16_NP)

    def pmajor(w):  # (HID, C) -> (128, KT, C)
        return np.ascontiguousarray(
            w.reshape(KT, 128, w.shape[1]).transpose(1, 0, 2)
        )

    in_maps = []
    for c in range(N_CORES):
        in_maps.append(
            {
                "xC": xC,
                "wq": pmajor(w_q[:, c * 512:(c + 1) * 512]),
                "wk": pmajor(w_k[:, c * D:(c + 1) * D]),
                "wv": pmajor(w_v[:, c * D:(c + 1) * D]),
                # wo rows (512, HID) -> (128, NH, HID)
                "wo": np.ascontiguousarray(
                    w_o[c * 512:(c + 1) * 512, :]
                    .reshape(NH, 128, HID)
                    .transpose(1, 0, 2)
                ),
                "cosF": cosF,
                "sinF": sinF,
                "swapP": swapP,
                "ident": ident,
                "maskT": maskT,
            }
        )
    return in_maps


def kernel(x, cos_half, sin_half, w_q, w_k, w_v, w_o, trace=None):
    if trace is None:
        trace = os.environ.get("KTRACE", "0") == "1"
    global LAST_RESULT
    in_maps = prep_inputs(x, cos_half, sin_half, w_q, w_k, w_v, w_o)
    res = run_bass_kernel_spmd(
        _nc(), in_maps, core_ids=list(range(N_CORES)), trace=trace
    )
    LAST_RESULT = res
    acc = res.results[0]["outT"].astype(np.float32)
    for c in range(1, N_CORES):
        acc += res.results[c]["outT"].astype(np.float32)
    return np.ascontiguousarray(acc.T).astype(BF16_NP).reshape(B, S, HID)


_NC = None
LAST_RESULT = None


def _nc():
    global _NC
    if _NC is None:
        _NC = build()
    return _NC


# revision 45
# speedup vs baseline: 1.2233x; 1.2233x over previous
"""Trainium2 Bass kernel for nn_Attention_27994596836196.

GQA attention block (B=2, S=2048, HID=4096, 32 q heads / 8 kv groups,
rope, causal, out-projection), tensor-parallel over the 8 NeuronCores of
one TRN2 chip: core c owns q heads 4c..4c+3 and kv group c.  Each core
computes its heads' Q^T/K^T/V projections from a host-pretransposed
activation matrix, runs causal attention in a transposed
(keys-on-partitions) layout, and contracts its 512-row slice of w_o into
a full-size partial output; the host sums the eight partials
(collectives deliberately avoided: a collective in the NEFF measurably
slows every PE instruction by ~20%).

v2 optimizations over the 849us baseline:
  - softmax denominator accumulated on DVE (tensor_add over exp tiles)
    instead of ones-matmuls on PE (-70us PE work)
  - causal diagonal tiles stream only their valid column range
  - V transposed on-chip via PE identity-matmuls (no DRAM roundtrip, no
    DMA_TRANSPOSE serialization)
  - software pipelining: b0 attention interleaves with b1 projection
    granules; b1 attention interleaves with b0 w_o emission, so PE has
    matmul filler while ACT runs exps
  - startup DMAs staggered (x chunk 0 in quarters, K projection first)
  - w_o PSUM evictions rotate across DVE/ACT/Pool engines

Self-contained: builds and runs via concourse (bass/tile) from
/opt/trn_rl_repo through bass_utils.run_bass_kernel_spmd on cores 0-7.
"""

import os
import sys
from collections import deque

sys.path.insert(0, "/opt/trn_rl_repo")

import numpy as np
import ml_dtypes

# NTFF profiling hook shim: this agent image's antenv package lacks
# axon_hooks, which run_bass_kernel_spmd(trace=True) imports.  Harmless
# when tracing is off; registers the real hook when available.
try:
    import antenv.axon_hooks  # noqa: F401
except ImportError:
    import types

    _m = types.ModuleType("antenv.axon_hooks")
    _m._HOOK = None
    _m.set_axon_ntff_profile_hook = lambda h: setattr(_m, "_HOOK", h)
    _m.get_axon_ntff_profile_hook = lambda: _m._HOOK
    sys.modules["antenv.axon_hooks"] = _m
    try:
        import antenv

        antenv.axon_hooks = _m
        from trn_agent_boot.trn_boot import _ntff_profile_via_ctypes

        _m.set_axon_ntff_profile_hook(
            _ntff_profile_via_ctypes("/opt/axon/libaxon_pjrt.so")
        )
    except Exception:
        pass

import bass_rust
import concourse.bass as bass
import concourse.tile as tile
from concourse import mybir
from concourse.bass_utils import run_bass_kernel_spmd
from contextlib import ExitStack

# ---------------------------------------------------------------------------
# Workaround for this walrus build's cap of ONE sync-wait command per
# instruction: Tile's sem-assignment freely attaches several waits to one
# instruction and codegen rejects it ("Too many sync wait commands").
# Split the waits across same-engine NoOps preceding the instruction.
# ---------------------------------------------------------------------------
MAX_WAITS = 1


def split_multi_waits(nc):
    n_split = 0
    for f in nc.m.functions:
        for bb in f.blocks:
            out = []
            for inst in bb.instructions:
                si = inst.sync_info
                if si is not None and si.on_wait and len(si.on_wait) > MAX_WAITS:
                    waits = list(si.on_wait)
                    extra, keep = waits[:-MAX_WAITS], waits[-MAX_WAITS:]
                    for i in range(0, len(extra), MAX_WAITS):
                        nop = bass_rust.InstNoOp(
                            name=f"I-{nc.next_id()}", ins=[], outs=[]
                        )
                        nop.engine = inst.engine
                        nop.sync_info = mybir.SyncInfo(
                            on_wait=extra[i : i + MAX_WAITS], on_update=[]
                        )
                        out.append(nop)
                    si.on_wait = keep
                    n_split += 1
                out.append(inst)
            bb.instructions[:] = out
    return n_split


BF16 = mybir.dt.bfloat16
F32 = mybir.dt.float32

N_CORES = 8
B, S, HID = 2, 2048, 4096
BS = B * S  # 4096
D = 128
NH = 4          # q heads per core
KT = HID // 128  # 32 k-tiles
SC = 512        # free-dim chunk
NSC = BS // SC  # 8
SCALE = 1.0 / (D ** 0.5)
EXP = mybir.ActivationFunctionType.Exp
LOG = mybir.ActivationFunctionType.Ln

# projection target order: K first (smallest weight DMA gates startup),
# then the 4 Q heads, V last (V has no rope; its eviction transposes).
TGT = [4, 0, 1, 2, 3, 5]


def build():
    nc = bass.Bass(num_devices=N_CORES)

    # all inputs host-prearranged to partition-major layouts so every DMA
    # is 128 large contiguous descriptors (descriptor-rate gated startup
    # otherwise: gather loads cost 2-9us EACH to issue)
    xC = nc.declare_dram_parameter("xC", [128, NSC, KT, SC], BF16, isOutput=False)
    wq = nc.declare_dram_parameter("wq", [128, KT, NH * D], BF16, isOutput=False)
    wk = nc.declare_dram_parameter("wk", [128, KT, D], BF16, isOutput=False)
    wv = nc.declare_dram_parameter("wv", [128, KT, D], BF16, isOutput=False)
    wo = nc.declare_dram_parameter("wo", [128, NH, HID], BF16, isOutput=False)
    cosF = nc.declare_dram_parameter("cosF", [D, BS], BF16, isOutput=False)
    sinF = nc.declare_dram_parameter("sinF", [D, BS], BF16, isOutput=False)
    swapP = nc.declare_dram_parameter("swapP", [D, D], BF16, isOutput=False)
    ident = nc.declare_dram_parameter("ident", [D, D], BF16, isOutput=False)
    maskT = nc.declare_dram_parameter("maskT", [D, D], BF16, isOutput=False)
    outT = nc.declare_dram_parameter("outT", [HID, BS], BF16, isOutput=True)

    with tile.TileContext(nc, num_cores=N_CORES) as tc, ExitStack() as ctx:
        # ---- long-lived pools -------------------------------------------
        singles = ctx.enter_context(tc.tile_pool(name="singles", bufs=1))
        qkv = ctx.enter_context(tc.tile_pool(name="qkv", bufs=1))
        pexp = ctx.enter_context(tc.tile_pool(name="pexp", bufs=8))
        lpool = ctx.enter_context(tc.tile_pool(name="lp", bufs=2))
        rlg = ctx.enter_context(tc.tile_pool(name="rlg", bufs=1))
        rrec = ctx.enter_context(tc.tile_pool(name="rrec", bufs=2))
        # PSUM: 8 banks of [128,512]f32 exactly:
        ps_p = ctx.enter_context(tc.tile_pool(name="ps_p", bufs=2, space="PSUM"))
        ps_s = ctx.enter_context(tc.tile_pool(name="ps_s", bufs=2, space="PSUM"))
        ps_o = ctx.enter_context(tc.tile_pool(name="ps_o", bufs=2, space="PSUM"))
        ps_x = ctx.enter_context(tc.tile_pool(name="ps_x", bufs=2, space="PSUM"))

        q_sb = [
            [
                qkv.tile([D, S], BF16, tag=f"q{b}_{h}", name=f"q_sb{b}_{h}")
                for h in range(NH)
            ]
            for b in range(B)
        ]
        k_sb = [qkv.tile([D, S], BF16, tag=f"k{b}", name=f"k_sb{b}") for b in range(B)]
        v_sb = [qkv.tile([D, S], BF16, tag=f"v{b}", name=f"v_sb{b}") for b in range(B)]

        # =================================================================
        # projection closures (stage 1 runs them inline; stage 2 pulls
        # them as PE filler between attention bursts)
        # =================================================================
        aall0 = ctx.enter_context(tc.tile_pool(name="aall0", bufs=1))
        ctx1 = ctx.enter_context(ExitStack())
        w1 = ctx1.enter_context(tc.tile_pool(name="w1", bufs=1))
        xtp = ctx1.enter_context(tc.tile_pool(name="xt", bufs=5))
        rope = ctx1.enter_context(tc.tile_pool(name="rope", bufs=3))
        vtp = ctx1.enter_context(tc.tile_pool(name="vt", bufs=2))

        wk_g = [
            w1.tile([128, 8, D], BF16, tag=f"wk{g}", name=f"wk_g{g}")
            for g in range(4)
        ]
        wv_sb = w1.tile([128, KT, D], BF16, tag="wv")
        wq_g = [
            w1.tile([128, 8, NH * D], BF16, tag=f"wq{g}", name=f"wq_g{g}")
            for g in range(4)
        ]
        cos_sb = singles.tile([D, BS], BF16)
        sin_sb = singles.tile([D, BS], BF16)
        mask_sb = singles.tile([D, D], BF16)
        swap_sb = singles.tile([D, D], BF16)
        ident_sb = singles.tile([D, D], BF16)
        ones_sb = singles.tile([D, D], BF16)
        nc.vector.memset(ones_sb[:], 1.0)

        def load_weights_early():
            # issued AFTER the first x piece + wk (HBM bw is the startup
            # gate: 8 cores pull simultaneously)
            nc.sync.dma_start(wq_g[0][:], wq[:, 0:8, :])
            nc.sync.dma_start(swap_sb[:], swapP[:])
            nc.sync.dma_start(ident_sb[:], ident[:])
            nc.sync.dma_start(mask_sb[:], maskT[:])
            nc.sync.dma_start(wq_g[1][:], wq[:, 8:16, :])
            nc.sync.dma_start(wq_g[2][:], wq[:, 16:24, :])
            nc.sync.dma_start(wq_g[3][:], wq[:, 24:32, :])
            nc.sync.dma_start(wv_sb[:], wv[:])
            nc.sync.dma_start(cos_sb[:], cosF[:])
            nc.sync.dma_start(sin_sb[:], sinF[:])

        pstate = {"pending": None, "xts": {}, "ps": {}}

        def rope_b(qeo, dst, sc):
            cols = bass.ts(sc, SC)
            loc = bass.ts(sc % 4, SC)
            ps_sw = ps_x.tile([D, SC], F32, tag="aux")
            nc.tensor.matmul(ps_sw[:], swap_sb[:], qeo[:], start=True, stop=True)
            t1 = rope.tile([D, SC], BF16, tag="t1")
            nc.vector.tensor_mul(t1[:], qeo[:], cos_sb[:, cols])
            t2 = rope.tile([D, SC], BF16, tag="t2")
            nc.vector.tensor_mul(t2[:], ps_sw[:], sin_sb[:, cols])
            nc.vector.tensor_add(dst[:, loc], t1[:], t2[:])

        def proj_dma(sc):
            ts = []
            for g in range(4):
                t = xtp.tile([128, 8, SC], BF16, tag="xt", name=f"xq{sc}_{g}")
                nc.sync.dma_start(t[:], xC[:, sc, g * 8 : (g + 1) * 8, :])
                ts.append(t)
            pstate["xts"][sc] = ts

        def proj_half(sc, ti, half):
            tgt = TGT[ti]
            xth = pstate["xts"][sc]
            if half == 0:
                pstate["ps"][(sc, ti)] = ps_t = ps_p.tile(
                    [D, SC], F32, tag="pp", name=f"pj{sc}_{ti}"
                )
            else:
                ps_t = pstate["ps"][(sc, ti)]
            for k in range(half * 16, half * 16 + 16):
                if tgt == 4:
                    lhs = wk_g[k // 8][:, k % 8, :]
                elif tgt == 5:
                    lhs = wv_sb[:, k, :]
                else:
                    lhs = wq_g[k // 8][:, k % 8, bass.ts(tgt, D)]
                nc.tensor.matmul(
                    ps_t[:], lhs, xth[k // 8][:, k % 8, :],
                    start=(k == 0), stop=(k == KT - 1),
                )
            if half == 0:
                return
            b = sc // 4
            if tgt != 5:
                qeo = rope.tile([D, SC], BF16, tag="qeo")
                nc.vector.tensor_copy(qeo[:], ps_t[:])
                if pstate["pending"] is not None:
                    rope_b(*pstate["pending"])
                dst = k_sb[b] if tgt == 4 else q_sb[b][tgt]
                pstate["pending"] = (qeo, dst, sc)
            else:
                # V: PSUM -> SBUF (v^T), 4 PE transposes, Pool evict to
                # natural [sk, d] layout
                vt = vtp.tile([D, SC], BF16, tag="vt")
                nc.vector.tensor_copy(vt[:], ps_t[:])
                tp = ps_x.tile([D, SC], BF16, tag="aux", name=f"vt{sc}")
                for u in range(4):
                    nc.tensor.transpose(
                        tp[:, bass.ts(u, D)], vt[:, bass.ts(u, D)], ident_sb[:]
                    )
                nc.vector.tensor_copy(v_sb[b][:, bass.ts(sc % 4, SC)], tp[:])
                if pstate["pending"] is not None:
                    rope_b(*pstate["pending"])
                    pstate["pending"] = None

        def proj_closures(scs, skip_first_dma=False, interleave_first=False):
            cl = deque()
            for i, sc in enumerate(scs):
                if i == 0 and not skip_first_dma:
                    cl.append(lambda sc=sc: proj_dma(sc))
                if i == 0 and interleave_first:
                    # consume x quarters as they arrive: K and Q0 halves
                    # interleaved (2 PSUM groups open, within pp's 2 bufs)
                    order = [(0, 0), (1, 0), (0, 1), (1, 1)] + [
                        (ti, half) for ti in range(2, 6) for half in range(2)
                    ]
                else:
                    order = [(ti, half) for ti in range(6) for half in range(2)]
                for ti, half in order:
                    if i + 1 < len(scs) and ti == 0 and half == 1:
                        def withdma(sc=sc, ti=ti, half=half, nx=scs[i + 1]):
                            proj_half(sc, ti, half)
                            proj_dma(nx)
                        cl.append(withdma)
                    else:
                        cl.append(
                            lambda sc=sc, ti=ti, half=half: proj_half(
                                sc, ti, half
                            )
                        )
            return cl

        # =================================================================
        # attention unit (b, h, c): 512 queries vs all causal keys.
        # keys-on-partitions layout; diagonal k-tiles stream only their
        # valid q-range; denominator accumulated on DVE; filler() is
        # pulled between QK quads to keep PE busy while ACT runs exps.
        # =================================================================
        evcnt = [0]

        def attn_unit(b, h, c, filler):
            nsk = 4 * (c + 1)
            j_seq = [(4 * c + dd, D * dd) for dd in range(4)] + [
                (j, 0) for j in range(4 * c)
            ]
            qh = q_sb[b][h]
            l_acc = lpool.tile([D, SC], BF16, tag="l")
            o_ps = ps_o.tile([D, SC], F32, tag="o")

            def av(idx, j, off, p):
                nc.tensor.matmul(
                    o_ps[:, off:],
                    v_sb[b][:, bass.ts(j, D)],
                    p[:, off:],
                    start=(idx == 0), stop=(idx == nsk - 1),
                    skip_group_check=(c == 0),
                )

            pend = deque()
            for idx, (j, off) in enumerate(j_seq):
                s_ps = ps_s.tile([D, SC], F32, tag="s")
                nc.tensor.matmul(
                    s_ps[:, off:],
                    k_sb[b][:, bass.ts(j, D)],
                    qh[:, c * SC + off : (c + 1) * SC],
                    start=True, stop=True,
                )
                p = pexp.tile([D, SC], BF16, tag="p")
                nc.scalar.activation(p[:, off:], s_ps[:, off:], EXP, scale=SCALE)
                if idx < 4:
                    nc.vector.tensor_mul(
                        p[:, off : off + D], p[:, off : off + D], mask_sb[:]
                    )
                if idx == 0:
                    nc.vector.tensor_copy(l_acc[:], p[:])
                else:
                    nc.vector.tensor_add(
                        l_acc[:, off:], l_acc[:, off:], p[:, off:]
                    )
                pend.append((idx, j, off, p))
                if len(pend) > 5:
                    av(*pend.popleft())
                if idx % 4 == 3:
                    filler()
            while pend:
                av(*pend.popleft())
            # partition-reduce the per-partition partial sums into the true
            # denominator, broadcast across partitions (single 512-stream
            # matmul per 512-query chunk instead of one per k-tile)
            l_bc = ps_s.tile([D, SC], F32, tag="s", name=f"lbc{b}_{h}_{c}")
            nc.tensor.matmul(l_bc[:], ones_sb[:], l_acc[:], start=True, stop=True)
            lg = rlg.tile([D, SC], F32, tag="lg")
            nc.scalar.activation(lg[:], l_bc[:], LOG)
            rec = rrec.tile([D, SC], F32, tag="rec")
            nc.scalar.activation(rec[:], lg[:], EXP, scale=-1.0)
            nc.vector.tensor_mul(
                a_all[b][h][:, bass.ts(c, SC)], o_ps[:], rec[:]
            )
            filler()

        # ---- w_o emission ------------------------------------------------
        def wo_pair(b, nl, m0, dve_only=False):
            for m in (m0, m0 + 1):
                pool, tg = (ps_p, "pp") if m % 2 == 0 else (ps_x, "aux")
                o_ps = pool.tile([D, SC], F32, tag=tg, name=f"wo{b}_{nl}_{m}")
                for hh in range(NH):
                    nc.tensor.matmul(
                        o_ps[:],
                        wo_h[m // 16][:, hh, bass.ts(m % 16, D)],
                        a_all[b][hh][:, bass.ts(nl, SC)],
                        start=(hh == 0), stop=(hh == NH - 1),
                    )
                ot = o3p.tile([D, SC], BF16, tag="ot")
                e = evcnt[0] = evcnt[0] + 1
                # ACT evictions only when no attention is interleaved —
                # they otherwise delay the exp chain and stall AV matmuls
                if dve_only or e % 2 == 0:
                    nc.vector.tensor_copy(ot[:], o_ps[:])
                else:
                    nc.scalar.copy(ot[:], o_ps[:])
                nc.sync.dma_start(
                    outT[bass.ts(m, D), b * S + nl * SC : b * S + (nl + 1) * SC],
                    ot[:],
                )

        a_all = [
            [
                aall0.tile([D, S], BF16, tag=f"a0_{h}", name=f"a_all0_{h}")
                for h in range(NH)
            ],
            None,  # batch-1 tiles allocated at stage 3 (SBUF lifetime)
        ]

        def make_pull(cl):
            def pull():
                if cl:
                    cl.popleft()()
            return pull

        # ---- stage 1: projections for batch 0 (sc 0-3) ------------------
        # first x quarter + wk lead the DMA queue: they gate the first
        # matmul, everything else streams behind them
        ts0 = [
            xtp.tile([128, 8, SC], BF16, tag="xt", name=f"xq0_{g}")
            for g in range(4)
        ]
        pstate["xts"][0] = ts0
        nc.sync.dma_start(ts0[0][:, 0:2, :], xC[:, 0, 0:2, :])
        nc.sync.dma_start(wk_g[0][:], wk[:, 0:8, :])
        nc.sync.dma_start(ts0[0][:, 2:8, :], xC[:, 0, 2:8, :])
        nc.sync.dma_start(wk_g[1][:], wk[:, 8:16, :])
        nc.sync.dma_start(ts0[1][:], xC[:, 0, 8:16, :])
        nc.sync.dma_start(wk_g[2][:], wk[:, 16:24, :])
        nc.sync.dma_start(wk_g[3][:], wk[:, 24:32, :])
        nc.sync.dma_start(ts0[2][:], xC[:, 0, 16:24, :])
        nc.sync.dma_start(ts0[3][:], xC[:, 0, 24:32, :])
        load_weights_early()
        for fn in proj_closures([0, 1, 2, 3], skip_first_dma=True, interleave_first=True):
            fn()

        # ---- stage 2: b1 projections interleaved with b0 attention ------
        cl2 = proj_closures([4, 5, 6, 7])
        pull2 = make_pull(cl2)
        for h in range(NH):
            for c in range(4):
                attn_unit(0, h, c, pull2)
        while cl2:
            pull2()
        ctx1.close()  # free w1/xt/rope/vt SBUF for stage 3

        # ---- stage 3: b1 attention interleaved with b0 w_o --------------
        w3 = ctx.enter_context(tc.tile_pool(name="w3", bufs=1))
        aall1 = ctx.enter_context(tc.tile_pool(name="aall1", bufs=1))
        o3p = ctx.enter_context(tc.tile_pool(name="o3p", bufs=6))
        a_all[1] = [
            aall1.tile([D, S], BF16, tag=f"a1_{h}", name=f"a_all1_{h}")
            for h in range(NH)
        ]
        # two halves so the first w_o fillers wait on 2MB, not 4MB
        wo_h = [
            w3.tile([128, NH, HID // 2], BF16, tag=f"wo{i}", name=f"wo_h{i}")
            for i in range(2)
        ]
        nc.gpsimd.dma_start(wo_h[0][:], wo[:, :, 0 : HID // 2])
        nc.gpsimd.dma_start(wo_h[1][:], wo[:, :, HID // 2 : HID])

        cl3 = deque()
        for nl in range(4):
            for m0 in range(0, KT, 2):
                cl3.append(lambda nl=nl, m0=m0: wo_pair(0, nl, m0))
        pull3 = make_pull(cl3)
        for h in range(NH):
            for c in range(4):
                attn_unit(1, h, c, pull3)
        while cl3:
            pull3()

        # ---- stage 4: b1 w_o --------------------------------------------
        for nl in range(4):
            for m0 in range(0, KT, 2):
                wo_pair(1, nl, m0)

    split_multi_waits(nc)
    return nc


BF16_NP = ml_dtypes.bfloat16


def prep_inputs(x, cos_half, sin_half, w_q, w_k, w_v, w_o):
    x = np.asarray(x)
    cos_half = np.asarray(cos_half, dtype=np.float32)
    sin_half = np.asarray(sin_half, dtype=np.float32)
    w_q, w_k, w_v, w_o = (np.asarray(a) for a in (w_q, w_k, w_v, w_o))

    X = x.reshape(B * S, HID)
    # partition-major x: xC[p, sc, k, s'] = X[sc*512+s', k*128+p]
    xC = np.ascontiguousarray(
        X.reshape(NSC, SC, KT, 128).transpose(3, 0, 2, 1)
    )

    cosb = cos_half.astype(BF16_NP)  # reference casts cos/sin to bf16 in _rope
    sinb = sin_half.astype(BF16_NP)
    cosF = np.tile(np.repeat(cosb.T, 2, axis=0), (1, B))  # (128, BS)
    sign = np.where(np.arange(D) % 2 == 0, -1.0, 1.0).astype(np.float32)
    sinF = np.tile(np.repeat(sinb.T, 2, axis=0) * sign[:, None].astype(BF16_NP), (1, B))
    cosF = np.ascontiguousarray(cosF, dtype=BF16_NP)
    sinF = np.ascontiguousarray(sinF, dtype=BF16_NP)

    swapP = np.zeros((D, D), dtype=BF16_NP)
    for i in range(D):
        swapP[i, i ^ 1] = 1.0

    ident = np.eye(D, dtype=BF16_NP)
    # strip mask for diagonal k-tiles: valid iff q_local >= k_local
    p = np.arange(D)[:, None]
    f = np.arange(D)[None, :]
    maskT = (f >= p).astype(BF16_NP)

    def pmajor(w):  # (HID, C) -> (128, KT, C)
        return np.ascontiguousarray(
            w.reshape(KT, 128, w.shape[1]).transpose(1, 0, 2)
        )

    in_maps = []
    for c in range(N_CORES):
        in_maps.append(
            {
                "xC": xC,
                "wq": pmajor(w_q[:, c * 512:(c + 1) * 512]),
                "wk": pmajor(w_k[:, c * D:(c + 1) * D]),
                "wv": pmajor(w_v[:, c * D:(c + 1) * D]),
                # wo rows (512, HID) -> (128, NH, HID)
                "wo": np.ascontiguousarray(
                    w_o[c * 512:(c + 1) * 512, :]
                    .reshape(NH, 128, HID)
                    .transpose(1, 0, 2)
                ),
                "cosF": cosF,
                "sinF": sinF,
                "swapP": swapP,
                "ident": ident,
                "maskT": maskT,
            }
        )
    return in_maps


def kernel(x, cos_half, sin_half, w_q, w_k, w_v, w_o, trace=None):
    if trace is None:
        trace = os.environ.get("KTRACE", "0") == "1"
    global LAST_RESULT
    in_maps = prep_inputs(x, cos_half, sin_half, w_q, w_k, w_v, w_o)
    res = run_bass_kernel_spmd(
        _nc(), in_maps, core_ids=list(range(N_CORES)), trace=trace
    )
    LAST_RESULT = res
    acc = res.results[0]["outT"].astype(np.float32)
    for c in range(1, N_CORES):
        acc += res.results[c]["outT"].astype(np.float32)
    return np.ascontiguousarray(acc.T).astype(BF16_NP).reshape(B, S, HID)


_NC = None
LAST_RESULT = None


def _nc():
    global _NC
    if _NC is None:
        _NC = build()
    return _NC
